# revision 27
# baseline (speedup 1.0000x reference)
"""DistLoss kernel for Trainium2, data-parallel over batch across 8 NeuronCores.

Per core b: computes, for each stroke row l, the top-16 most-similar pixels of
ref image b (by color distance to the sampled "pooled" color), then the min
squared distance from pos[b, l+1] to those 16 pixel locations. Host averages.

Self-contained: hardcoded shapes for predictions (8,256,8), ref_imgs (8,3,192,192).
"""
import numpy as np
from contextlib import ExitStack

import concourse.bass as bass
import concourse.mybir as mybir
from concourse.bass_utils import run_bass_kernel_spmd

F32 = mybir.dt.float32
U32 = mybir.dt.uint32

BS = 8
L = 256
CH = 3
H = W = 192
HW = H * W            # 36864
K = 16
CHUNK = 16
NCHUNK = HW // CHUNK  # 2304
NG = 2                # row groups of 128
SUP = 2048            # supertile cols (4 matmuls x 512)
NSUP = HW // SUP      # 18 per group
SIXTH = HW // 6       # 6144 (rhs staging column blocks)
NEG_BIG = -1e30
BIG = 1e30
INV192 = float(np.float32(1.0) / np.float32(192.0))
MAGIC = 8388608.0     # 2^23 round-to-int trick

_CACHE = {}


def _build(debug=False):
    kstage = 4
    nc = bass.Bass()
    img_in = nc.declare_dram_parameter("img", [CH, H, W], F32, isOutput=False)
    lhsT_in = nc.declare_dram_parameter("lhsT", [4, 256], F32, isOutput=False)
    w_ins = [nc.declare_dram_parameter(f"w{c}{g}", [128, 1], F32, isOutput=False)
             for c in range(3) for g in range(NG)]
    qx0_in = nc.declare_dram_parameter("qx0", [128, 1], F32, isOutput=False)
    qx1_in = nc.declare_dram_parameter("qx1", [128, 1], F32, isOutput=False)
    qy0_in = nc.declare_dram_parameter("qy0", [128, 1], F32, isOutput=False)
    qy1_in = nc.declare_dram_parameter("qy1", [128, 1], F32, isOutput=False)
    iop1_in = nc.declare_dram_parameter("iop1", [128, NCHUNK], F32, isOutput=False)
    jt_in = nc.declare_dram_parameter("jt", [128, 256], F32, isOutput=False)

    terms_out = nc.declare_dram_parameter("terms", [NG, 128], F32, isOutput=True)
    if debug:
        cm0_out = nc.declare_dram_parameter("cm0", [128, NCHUNK], F32, isOutput=True)
        ci0_out = nc.declare_dram_parameter("ci0", [128, 16], F32, isOutput=True)
        off0_out = nc.declare_dram_parameter("off0", [128, 16], U32, isOutput=True)
        sg0_out = nc.declare_dram_parameter("sg0", [128, 256], F32, isOutput=True)
        dm0_out = nc.declare_dram_parameter("dm0", [128, 256], F32, isOutput=True)
        sdbg_out = nc.declare_dram_parameter("sdbg", [128, 1024], F32, isOutput=True)

    ctab = nc.dram_tensor("ctab", [NCHUNK, 64], F32)
    rsq_dram = nc.dram_tensor("rsq_dram", [HW], F32)

    es = ExitStack()
    ec = es.enter_context
    # staging for refsq compute: pixel = p*384 + j
    A = ec(nc.sbuf_tensor([96, 3 * 384], F32))
    t0 = ec(nc.sbuf_tensor([96, 384], F32))
    t1 = ec(nc.sbuf_tensor([96, 384], F32))
    rsq = ec(nc.sbuf_tensor([96, 384], F32))
    # rhs staging: sixth q lives in R_AB[q//3] at base partition 32*(q%3);
    # rows base+0..2 = ref channels, base+3 = refsq (matmul needs base 0/32/64)
    R_AB = [ec(nc.sbuf_tensor(f"Rab{i}", [68, SIXTH], F32)) for i in range(2)]
    lhsT = ec(nc.sbuf_tensor([68, 256], F32))
    CM = [ec(nc.sbuf_tensor(f"CM{i}", [128, NCHUNK], F32)) for i in range(NG)]
    CMt = ec(nc.sbuf_tensor([128, NCHUNK], F32))
    iop1 = ec(nc.sbuf_tensor([128, NCHUNK], F32))
    maskf = ec(nc.sbuf_tensor([128, NCHUNK], F32))
    selv = ec(nc.sbuf_tensor([128, NCHUNK], F32))
    selt = ec(nc.sbuf_tensor([128, NCHUNK], F32))
    u1 = ec(nc.sbuf_tensor([128, 8], F32))
    u2 = ec(nc.sbuf_tensor([128, 8], F32))
    thr = ec(nc.sbuf_tensor([128, 1], F32))
    ci = [ec(nc.sbuf_tensor(f"cisb{i}", [128, 16], F32)) for i in range(NG)]
    offs = [ec(nc.sbuf_tensor(f"offssb{i}", [128, 16], U32)) for i in range(NG)]
    offs_f = ec(nc.sbuf_tensor([128, 16], F32))
    wsc = [[ec(nc.sbuf_tensor(f"wsb{c}{g}", [128, 1], F32)) for g in range(NG)]
           for c in range(3)]
    qx = [ec(nc.sbuf_tensor(f"qxsb{i}", [128, 1], F32)) for i in range(NG)]
    qy = [ec(nc.sbuf_tensor(f"qysb{i}", [128, 1], F32)) for i in range(NG)]
    SG = [ec(nc.sbuf_tensor(f"SG{i}", [128, 256], F32)) for i in range(NG)]
    SGR = [ec(nc.sbuf_tensor(f"SGR{i}", [128, 1024], F32)) for i in range(NG)]
    SGt = ec(nc.sbuf_tensor([128, 256], F32))
    jt = ec(nc.sbuf_tensor([128, 256], F32))
    # tail temps
    maskc = ec(nc.sbuf_tensor([128, 256], F32))
    cie = ec(nc.sbuf_tensor([128, 256], F32))
    px = ec(nc.sbuf_tensor([128, 256], F32))
    qr = ec(nc.sbuf_tensor([128, 256], F32))
    r0 = ec(nc.sbuf_tensor([128, 256], F32))
    lo = ec(nc.sbuf_tensor([128, 256], F32))
    hh = ec(nc.sbuf_tensor([128, 256], F32))
    ww = ec(nc.sbuf_tensor([128, 256], F32))
    dx = ec(nc.sbuf_tensor([128, 256], F32))
    dd = ec(nc.sbuf_tensor([128, 256], F32))
    dm = ec(nc.sbuf_tensor([128, 256], F32))
    dbg = ec(nc.sbuf_tensor([128, 32], F32))
    term = [ec(nc.sbuf_tensor(f"termsb{i}", [128, 1], F32)) for i in range(NG)]

    P = [ec(nc.psum_tensor(f"P{i}", [128, SUP], F32)) for i in range(2)]

    in_sem = ec(nc.semaphore("in_sem"))
    rsqd_sem = ec(nc.semaphore("rsqd_sem"))    # DVE refsq done
    rsqr_sem = ec(nc.semaphore("rsqr_sem"))    # relayout DMA done (x16 each, 6 DMAs)
    pe_sem = ec(nc.semaphore("pe_sem"))        # +1 per supertile
    red_sem = ec(nc.semaphore("red_sem"))      # +1 per supertile reduce
    tab_sem = ec(nc.semaphore("tab_sem"))      # +16 per ctab build DMA
    ci_sem = ec(nc.semaphore("ci_sem"))        # +1 per group extraction done
    gat_sem = ec(nc.semaphore("gat_sem"))      # +16 per gather DMA
    term_sem = ec(nc.semaphore("term_sem"))    # +1 per group tail done
    out_sem = ec(nc.semaphore("out_sem"))
    block = ec(nc.Block())

    # n input DMAs: A(1) + R24 ref(3) + lhsT(1) + rb/qx/qy(6) + iop1(1) + jt(1) = 13
    N_IN = 36

    @block.sync
    def _(sync):
        # inputs
        for c in range(3):
            sync.dma_start(
                out=A[:, c * 384:(c + 1) * 384],
                in_=img_in[c, :, :].rearrange("h w -> (h w)").rearrange(
                    "(p j) -> p j", j=384),
            ).then_inc(in_sem, 16)
        for x in range(2):
            for c in range(3):
                for m in range(3):
                    sync.dma_start(
                        out=R_AB[x][32 * m + c:32 * m + c + 1, :],
                        in_=img_in[c, :, :].rearrange("h w -> (h w)")[
                            (3 * x + m) * SIXTH:(3 * x + m + 1) * SIXTH],
                    ).then_inc(in_sem, 16)
        for base in (0, 32, 64):
            sync.dma_start(out=lhsT[base:base + 4, :],
                           in_=lhsT_in[:]).then_inc(in_sem, 16)
        w_pairs = [(wsc[c][g], w_ins[c * NG + g]) for c in range(3) for g in range(NG)]
        for sb, inp in w_pairs + [(qx[0], qx0_in),
                        (qx[1], qx1_in), (qy[0], qy0_in), (qy[1], qy1_in)]:
            sync.dma_start(out=sb[:], in_=inp[:]).then_inc(in_sem, 16)
        sync.dma_start(out=iop1[:], in_=iop1_in[:]).then_inc(in_sem, 16)
        sync.dma_start(out=jt[:], in_=jt_in[:]).then_inc(in_sem, 16)
        # refsq relayout: 6 DMAs, SBUF->SBUF partition flatten
        sync.wait_ge(rsqd_sem, 1)
        sync.dma_start(out=rsq_dram[:], in_=rsq[:]).then_inc(rsqr_sem, 16)
        sync.wait_ge(rsqr_sem, 16)
        for q in range(6):
            base = 32 * (q % 3)
            sync.dma_start(
                out=R_AB[q // 3][base + 3:base + 4, :],
                in_=rsq_dram[q * SIXTH:(q + 1) * SIXTH],
            ).then_inc(rsqr_sem, 16)
        # chunk table: [r0|r1|r2|rsq] per chunk of 16 pixels (DRAM->DRAM)
        for c in range(3):
            sync.dma_start(
                out=ctab[:, c * 16:(c + 1) * 16],
                in_=img_in[c, :, :].rearrange("h w -> (h w)").rearrange(
                    "(n k) -> n k", k=16),
            ).then_inc(tab_sem, 16)
        sync.dma_start(
            out=ctab[:, 48:64],
            in_=rsq_dram[:].rearrange("(n k) -> n k", k=16),
        ).then_inc(tab_sem, 16)

        # outputs
        for g in range(NG):
            sync.wait_ge(term_sem, g + 1)
            sync.dma_start(out=terms_out[g, :], in_=term[g][:]).then_inc(out_sem, 16)
        n_out = 2
        if debug:
            sync.dma_start(out=cm0_out[:], in_=CM[0][:]).then_inc(out_sem, 16)
            sync.dma_start(out=ci0_out[:], in_=ci[0][:]).then_inc(out_sem, 16)
            sync.dma_start(out=off0_out[:], in_=offs[0][:]).then_inc(out_sem, 16)
            sync.dma_start(out=sg0_out[:], in_=SG[0][:]).then_inc(out_sem, 16)
            sync.dma_start(out=dm0_out[:, 0:32], in_=dbg[:]).then_inc(out_sem, 16)
            sync.dma_start(out=dm0_out[:, 32:256], in_=dm[:, 32:256]).then_inc(out_sem, 16)
            sync.dma_start(out=sdbg_out[:], in_=SGR[0][:, 0:2048] if False else SGR[0][:, 0:1024]).then_inc(out_sem, 16)
            n_out += 7
        sync.wait_ge(out_sem, 16 * n_out)

    @block.tensor
    def _(tensor):
        tensor.wait_ge(in_sem, 16 * N_IN)
        tensor.wait_ge(rsqr_sem, 16 * 7)
        for i in range(NG * NSUP):
            g, s = divmod(i, NSUP)
            if i >= 2:
                tensor.wait_ge(red_sem, i - 1)
            c0 = s * SUP
            q, loc = divmod(c0, SIXTH)
            base = 32 * (q % 3)
            mm = None
            for k in range(4):
                mm = nc.tensor.matmul(
                    out=P[i % 2][:, k * 512:(k + 1) * 512],
                    lhsT=lhsT[base:base + 4, g * 128:(g + 1) * 128],
                    rhs=R_AB[q // 3][base:base + 4,
                                     loc + k * 512:loc + (k + 1) * 512],
                    start=True, stop=True,
                )
            mm.then_inc(pe_sem, 1)

    @block.vector
    def _(vector):
        # refsq = r0^2 + r1^2 + r2^2 on [96, 384] staging
        vector.wait_ge(in_sem, 16 * N_IN)
        nc.vector.tensor_tensor(t0[:], A[:, 0:384], A[:, 0:384],
                                op=mybir.AluOpType.mult)
        nc.vector.tensor_tensor(t1[:], A[:, 384:768], A[:, 384:768],
                                op=mybir.AluOpType.mult)
        nc.vector.tensor_tensor(t0[:], t0[:], t1[:], op=mybir.AluOpType.add)
        nc.vector.tensor_tensor(t1[:], A[:, 768:1152], A[:, 768:1152],
                                op=mybir.AluOpType.mult)
        nc.vector.tensor_tensor(rsq[:], t0[:], t1[:],
                                op=mybir.AluOpType.add).then_inc(rsqd_sem, 1)

        def extraction(g):
            # top-16 chunk values -> threshold -> winner chunk indices via iota
            nc.vector.max(u1[:], CM[g][:])
            nc.vector.tensor_copy(CMt[:], CM[g][:])
            nc.vector.match_replace(CMt[:], u1[:], CMt[:], NEG_BIG)
            nc.vector.max(u2[:], CMt[:])
            # spacers: small DVE writes land late; don't read u2 immediately
            nc.vector.tensor_copy(selt[:, 0:128], iop1[:, 0:128])
            nc.vector.tensor_copy(selt[:, 128:256], iop1[:, 128:256])
            if debug and g == 0:
                nc.vector.tensor_copy(dbg[:, 0:8], u1[:])
                nc.vector.tensor_copy(dbg[:, 8:16], u2[:])
            nc.vector.tensor_tensor(maskf[:], CM[g][:],
                                     u2[:, 7:8].to_broadcast([128, NCHUNK]),
                                     op=mybir.AluOpType.is_ge)
            nc.vector.tensor_tensor(selv[:], iop1[:], maskf[:],
                                    op=mybir.AluOpType.mult)
            nc.vector.tensor_scalar(selv[:], selv[:], -1.0, None,
                                    op0=mybir.AluOpType.add)
            nc.vector.max(ci[g][:, 0:8], selv[:])
            nc.vector.tensor_copy(selt[:], selv[:])
            nc.vector.match_replace(selt[:], ci[g][:, 0:8], selt[:], -1.0)
            nc.vector.max(ci[g][:, 8:16], selt[:])
            nc.vector.tensor_copy(maskf[:, 0:128], iop1[:, 0:128])  # spacer
            # offsets into ctab = chunk index directly
            nc.vector.tensor_copy(offs[g][:], ci[g][:]).then_inc(ci_sem, 1)

        def tail(g):
            # s" for candidates from gathered raw pixels:
            # SGR slot k cols k*64 + [r0(16)|r1(16)|r2(16)|rsq(16)]
            def part(c):
                return SGR[g][:].rearrange("p (a b) -> p a b", b=64)[:, :, c * 16:(c + 1) * 16]
            nc.vector.tensor_scalar(
                SG[g][:].rearrange("p (a b) -> p a b", b=16),
                part(0), wsc[0][g][:, 0:1], None, op0=mybir.AluOpType.mult)
            nc.vector.tensor_scalar(
                SGt[:].rearrange("p (a b) -> p a b", b=16),
                part(1), wsc[1][g][:, 0:1], None, op0=mybir.AluOpType.mult)
            nc.vector.tensor_tensor(SG[g][:], SG[g][:], SGt[:],
                                    op=mybir.AluOpType.add)
            nc.vector.tensor_scalar(
                SGt[:].rearrange("p (a b) -> p a b", b=16),
                part(2), wsc[2][g][:, 0:1], None, op0=mybir.AluOpType.mult)
            nc.vector.tensor_tensor(SG[g][:], SG[g][:], SGt[:],
                                    op=mybir.AluOpType.add)
            nc.vector.tensor_copy(SGt[:].rearrange("p (a b) -> p a b", b=16), part(3))
            nc.vector.tensor_tensor(SG[g][:], SG[g][:], SGt[:],
                                    op=mybir.AluOpType.subtract)
            # threshold = 16th largest of candidates
            nc.vector.max(u1[:], SG[g][:])
            nc.vector.tensor_copy(SGt[:], SG[g][:])
            nc.vector.match_replace(SGt[:], u1[:], SGt[:], NEG_BIG)
            nc.vector.max(u2[:], SGt[:])
            # candidate pixel index px = ci*16 + j  (also spaces out u2 write)
            nc.vector.tensor_copy(
                cie[:].rearrange("p (a b) -> p a b", b=16),
                ci[g][:, :, None].to_broadcast([128, 16, 16]),
            )
            nc.vector.tensor_scalar(px[:], cie[:], 16.0, None,
                                    op0=mybir.AluOpType.mult)
            nc.vector.tensor_tensor(px[:], px[:], jt[:], op=mybir.AluOpType.add)
            # h = px // 192 (round then fix), w = px - 192h
            nc.vector.tensor_scalar(qr[:], px[:], INV192, None,
                                    op0=mybir.AluOpType.mult)
            nc.vector.tensor_scalar(qr[:], qr[:], MAGIC, MAGIC,
                                    op0=mybir.AluOpType.add,
                                    op1=mybir.AluOpType.subtract)
            nc.vector.tensor_scalar(r0[:], qr[:], -192.0, None,
                                    op0=mybir.AluOpType.mult)
            nc.vector.tensor_tensor(r0[:], r0[:], px[:], op=mybir.AluOpType.add)
            nc.vector.tensor_scalar(lo[:], r0[:], 0.0, None,
                                    op0=mybir.AluOpType.is_lt)
            nc.vector.tensor_tensor(hh[:], qr[:], lo[:],
                                    op=mybir.AluOpType.subtract)
            nc.vector.tensor_scalar(ww[:], lo[:], 192.0, None,
                                    op0=mybir.AluOpType.mult)
            nc.vector.tensor_tensor(ww[:], ww[:], r0[:], op=mybir.AluOpType.add)
            nc.vector.tensor_tensor(maskc[:], SG[g][:],
                                     u2[:, 7:8].to_broadcast([128, 256]),
                                     op=mybir.AluOpType.is_ge)
            # d = (tx - qx)^2 + (ty - qy)^2
            nc.vector.tensor_scalar(ww[:], ww[:], INV192, None,
                                    op0=mybir.AluOpType.mult)
            nc.vector.tensor_scalar(hh[:], hh[:], INV192, None,
                                    op0=mybir.AluOpType.mult)
            nc.vector.tensor_scalar(dx[:], ww[:], qx[g][:, 0:1], None,
                                    op0=mybir.AluOpType.subtract)
            nc.vector.tensor_tensor(dd[:], dx[:], dx[:], op=mybir.AluOpType.mult)
            nc.vector.tensor_scalar(dx[:], hh[:], qy[g][:, 0:1], None,
                                    op0=mybir.AluOpType.subtract)
            nc.vector.tensor_tensor(dx[:], dx[:], dx[:], op=mybir.AluOpType.mult)
            nc.vector.tensor_tensor(dd[:], dd[:], dx[:], op=mybir.AluOpType.add)
            # dm = d + (1 - maskc) * BIG ; term = min
            nc.vector.tensor_scalar(dm[:], maskc[:], -1.0, -BIG,
                                    op0=mybir.AluOpType.add,
                                    op1=mybir.AluOpType.mult)
            nc.vector.tensor_tensor(dm[:], dm[:], dd[:], op=mybir.AluOpType.add)
            nc.vector.tensor_reduce(term[g][:], dm[:], op=mybir.AluOpType.min,
                                    axis=mybir.AxisListType.X).then_inc(term_sem, 1)

        for i in range(NG * NSUP):
            g, s = divmod(i, NSUP)
            vector.wait_ge(pe_sem, i + 1)
            rr = None
            for k in range(4):
                rr = nc.vector.tensor_reduce(
                    CM[g][:, s * 128 + k * 32:s * 128 + (k + 1) * 32],
                    P[i % 2][:, k * 512:(k + 1) * 512].rearrange(
                        "p (c k) -> p c k", k=CHUNK),
                    op=mybir.AluOpType.max,
                    axis=mybir.AxisListType.X,
                )
            rr.then_inc(red_sem, 1)
            if s == NSUP - 1 and kstage >= 2:
                extraction(g)
        for g in range(NG):
            if kstage >= 4:
                vector.wait_ge(gat_sem, 16 * 16 * (g + 1))
                tail(g)
            else:
                vector.wait_ge(red_sem, NSUP * (g + 1))
                if kstage >= 3:
                    vector.wait_ge(gat_sem, 16 * 16 * (g + 1))
                nc.vector.memset(term[g][:], 0.0).then_inc(term_sem, 1)

    @block.gpsimd
    def _(gpsimd):
        for g in range(NG if kstage >= 3 else 0):
            gpsimd.wait_ge(ci_sem, g + 1)
            gpsimd.wait_ge(tab_sem, 64)
            for k in range(16):
                gpsimd.indirect_dma_start(
                    out=SGR[g][:, k * 64:(k + 1) * 64],
                    out_offset=None,
                    in_=ctab[:],
                    in_offset=bass.IndirectOffsetOnAxis(
                        ap=offs[g][:, k:k + 1], axis=0),
                    bounds_check=NCHUNK - 1,
                    oob_is_err=False,
                ).then_inc(gat_sem, 16)

    es.close()
    return nc


def _host_prep(predictions, ref_imgs):
    """Mirror of the reference's pooled-color computation, plus per-core aux."""
    predictions = np.asarray(predictions, dtype=np.float32)
    ref_imgs = np.asarray(ref_imgs, dtype=np.float32)
    bs, Ln, _ = predictions.shape
    pos = predictions[:, :, :2]
    grid = pos.reshape(bs * Ln, 2)
    img_idx = np.arange(bs * Ln) % bs
    ix = np.round(grid[:, 0] * np.float32(W) - np.float32(0.5)).astype(np.int32)
    iy = np.round(grid[:, 1] * np.float32(H) - np.float32(0.5)).astype(np.int32)
    inb = ((ix >= 0) & (ix < W) & (iy >= 0) & (iy < H)).astype(np.float32)
    ixc = np.clip(ix, 0, W - 1)
    iyc = np.clip(iy, 0, H - 1)
    pooled_flat = ref_imgs[img_idx, :, iyc, ixc] * inb[:, None]
    pooled = pooled_flat.reshape(Ln, bs, CH).transpose(1, 0, 2)  # (bs, L, 3)

    iop1 = np.tile(np.arange(1, NCHUNK + 1, dtype=np.float32), (128, 1))
    jtile = np.tile(np.tile(np.arange(CHUNK, dtype=np.float32), 16), (128, 1))

    in_maps = []
    for b in range(bs):
        lhsT = np.empty((4, 256), dtype=np.float32)
        lhsT[0:3, :] = (2.0 * pooled[b]).T.astype(np.float32)
        lhsT[3, :] = -1.0
        q = np.concatenate([pos[b, 1:], pos[b, :1]], axis=0)  # (256, 2), row 255 dummy
        m = {
            "img": np.ascontiguousarray(ref_imgs[b]),
            "lhsT": lhsT,
            "qx0": np.ascontiguousarray(q[:128, 0]).reshape(128, 1),
            "qx1": np.ascontiguousarray(q[128:, 0]).reshape(128, 1),
            "qy0": np.ascontiguousarray(q[:128, 1]).reshape(128, 1),
            "qy1": np.ascontiguousarray(q[128:, 1]).reshape(128, 1),
            "iop1": iop1,
            "jt": jtile,
        }
        for c in range(3):
            for g in range(NG):
                m[f"w{c}{g}"] = np.ascontiguousarray(
                    2.0 * pooled[b, g * 128:(g + 1) * 128, c]).reshape(128, 1).astype(np.float32)
        in_maps.append(m)
    return in_maps


def _get_nc(debug=False):
    key = ("nc", debug)
    if key not in _CACHE:
        _CACHE[key] = _build(debug)
    return _CACHE[key]


def run_device(predictions, ref_imgs, debug=False, trace=False):
    nc = _get_nc(debug)
    in_maps = _host_prep(predictions, ref_imgs)
    res = run_bass_kernel_spmd(nc, in_maps, list(range(BS)), trace=trace)
    return res


def kernel(predictions, ref_imgs):
    res = run_device(predictions, ref_imgs)
    terms = np.stack([res.results[b]["terms"].reshape(256) for b in range(BS)])
    valid = terms[:, :255]  # term[l] pairs stroke l+1 with candidates of stroke l
    return np.float32(np.mean(valid.astype(np.float64)))


if __name__ == "__main__":
    rng = np.random.default_rng(0)
    p = rng.random((8, 256, 8), dtype=np.float32)
    r = rng.random((8, 3, 192, 192), dtype=np.float32)
    print(kernel(p, r))
